# revision 1
# baseline (speedup 1.0000x reference)
"""Clustered attention Trainium2 kernel (8-core SPMD, sharded along v).

Math (per batch b):
    sum_tot = key.sum(axis=2)                          # (L, D)
    S[i,k,j] = query[i,k,:] . sum_tot[j,:]
    A = softmax_j(scale * S  masked to label[i]==label[j])
    out[i,k,:] = sum_j A[i,k,j] * value[j,k,:]

Device layout (per (b, v) pair, v sharded 8 ways -> Vc=8 per core):
    S^T tiles (j on partitions, i free) = (sum_tot^T slice).T @ q^T,
    computed in float32r (full-rate fp32 matmul at N>=256).
    no-max softmax: A' = exp(scale*S) * mask01   (scores bounded ~|44|, safe in
    fp32/bf16 exponent range).
    value packed bf16 with a trailing ones column -> the A'^T @ [V|1] matmul
    yields both the numerator and the softmax denominator in one accumulation
    group. Normalize with a per-partition reciprocal multiply.
"""

import numpy as np
import ml_dtypes

import concourse.bass as bass
import concourse.tile as tile
from concourse import mybir
from concourse.bass import ts
from concourse.bass_utils import run_bass_kernel_spmd

BF16 = ml_dtypes.bfloat16
F32 = np.float32

# Problem shape (hardcoded per contract: kernel.py is self-contained).
B, L, V, D = 2, 512, 64, 128
N_CORES = 8
VC = V // N_CORES          # v slots per core
T = L // 128               # 128-row tiles along L
SCALE = 1.0 / float(np.sqrt(D))


# walrus's sync-wait lowering only tolerates 1 wait per instruction; Tile can
# emit more. Hoist the excess onto preceding same-engine NoOps (the engine
# sequencer performs waits in order, so semantics are unchanged).
_WAIT_EXEMPT = {
    "InstEventSemaphore", "InstNoOp", "InstCall", "InstISA",
    "InstUnconditionalBranch", "InstCompareAndBranch", "InstRegisterMove",
    "InstBranchHint", "InstHalt",
}


def _split_waits(nc, dma_cap=1, compute_cap=1):
    fn = nc.m.functions[0]
    for blk in fn.blocks:
        il = blk.instructions
        new = []
        changed = False
        for inst in il:
            tname = type(inst).__name__
            si = inst.sync_info
            if si is not None and tname not in _WAIT_EXEMPT:
                cap = dma_cap if tname in ("InstDMACopy", "InstDMA") else compute_cap
                waits = list(si.on_wait)
                if len(waits) > cap:
                    excess, keep = waits[:-cap], waits[-cap:]
                    for w in excess:
                        nop = mybir.InstNoOp(
                            name=nc.get_next_instruction_name(),
                            sync_info=mybir.SyncInfo(on_wait=[w], on_update=[]),
                            engine=inst.engine,
                            bass_nofuse=True,
                        )
                        new.append(nop)
                    inst.sync_info = mybir.SyncInfo(
                        on_wait=keep, on_update=list(si.on_update)
                    )
                    changed = True
            new.append(inst)
        if changed:
            blk.instructions = new


def _build_bass():
    nc = bass.Bass()
    bf = mybir.dt.bfloat16
    f32 = mybir.dt.float32
    f32r = mybir.dt.float32r

    qf = nc.dram_tensor("qf", (B, VC, D, L), f32r, kind="ExternalInput")
    sf = nc.dram_tensor("sf", (B, D, L), f32r, kind="ExternalInput")
    vp = nc.dram_tensor("vp", (B, VC, 128, T, D + 1), bf, kind="ExternalInput")
    mk = nc.dram_tensor("mk", (B, 128, T, L), bf, kind="ExternalInput")
    out = nc.dram_tensor("out", (B, L, VC, D), f32, kind="ExternalOutput")

    with tile.TileContext(nc) as tc:
        with (
            tc.tile_pool(name="consts", bufs=1) as cpool,
            tc.tile_pool(name="qin", bufs=6) as qpool,
            tc.tile_pool(name="vin", bufs=6) as vpool,
            tc.tile_pool(name="aw", bufs=4) as apool,
            tc.tile_pool(name="og", bufs=6) as opool,
            tc.tile_pool(name="rc", bufs=8) as rpool,
            tc.tile_pool(name="spsum", bufs=2, space="PSUM") as spsum,
            tc.tile_pool(name="opsum", bufs=4, space="PSUM") as opsum,
        ):
            sf_all = cpool.tile([128, B, L], f32r)
            mk_all = cpool.tile([128, B, T, L], bf)
            nc.sync.dma_start(out=sf_all, in_=sf[:, :, :].rearrange("b d l -> d b l"))
            nc.sync.dma_start(out=mk_all, in_=mk[:, :, :, :].rearrange("b p t l -> p b t l"))
            for b in range(B):
                sfb = sf_all[:, b, :]
                mkb = mk_all[:, b, :, :]
                for v in range(VC):
                    qt = qpool.tile([128, L], f32r)
                    nc.sync.dma_start(out=qt, in_=qf[b, v])
                    vv = vpool.tile([128, T, D + 1], bf)
                    nc.sync.dma_start(out=vv, in_=vp[b, v])

                    # A'^T, all four j-tiles: partitions = j%128, free = (jt, i)
                    at = apool.tile([128, T, L], bf)
                    for g in range(T // 2):
                        ps = spsum.tile([128, 2, L], f32)
                        for h in range(2):
                            jt = 2 * g + h
                            nc.tensor.matmul(
                                ps[:, h, :], sfb[:, ts(jt, 128)], qt,
                                start=True, stop=True,
                            )
                        nc.scalar.activation(
                            at[:, 2 * g:2 * g + 2, :], ps,
                            mybir.ActivationFunctionType.Exp, scale=SCALE,
                        )
                        nc.vector.tensor_mul(
                            at[:, 2 * g:2 * g + 2, :],
                            at[:, 2 * g:2 * g + 2, :],
                            mkb[:, 2 * g:2 * g + 2, :],
                        )

                    og = opool.tile([128, T, D], f32)
                    for it in range(T):
                        ops = opsum.tile([128, D + 1], f32)
                        for jt in range(T):
                            nc.tensor.matmul(
                                ops, at[:, jt, ts(it, 128)], vv[:, jt, :],
                                start=(jt == 0), stop=(jt == T - 1),
                            )
                        rc = rpool.tile([128, 1], f32)
                        nc.vector.reciprocal(rc, ops[:, D:D + 1])
                        nc.vector.tensor_scalar_mul(og[:, it, :], ops[:, 0:D], rc)
                    nc.sync.dma_start(
                        out=out[b, :, v, :].rearrange("(t p) d -> p t d", p=128),
                        in_=og,
                    )
    _split_waits(nc)
    return nc


_BASS_CACHE = None


def _get_bass():
    global _BASS_CACHE
    if _BASS_CACHE is None:
        _BASS_CACHE = _build_bass()
    return _BASS_CACHE


def _prepare_inputs(query, key, value, label_arr):
    """Host-side packing: transposes/casts + per-core sharding."""
    query = np.asarray(query, dtype=F32)
    key = np.asarray(key, dtype=F32)
    value = np.asarray(value, dtype=F32)
    lab = np.asarray(label_arr)

    sum_tot = key.sum(axis=2)                                 # (B, L, D) f32
    sT = np.ascontiguousarray(sum_tot.transpose(0, 2, 1))     # (B, D, L)
    qT = np.ascontiguousarray(query.transpose(0, 2, 3, 1))    # (B, V, D, L)

    # value packed (B, V, 128, T, D+1) with ones in the last column
    v4 = value.reshape(B, T, 128, V, D).transpose(0, 3, 2, 1, 4)  # (B,V,128,T,D)
    vp = np.empty((B, V, 128, T, D + 1), dtype=BF16)
    vp[..., :D] = v4.astype(BF16)
    vp[..., D] = np.ones((), dtype=BF16)

    # mask (B, 128, T, L): mask[b, jm, t, i] = lab[b, t*128+jm] == lab[b, i]
    labr = lab.reshape(B, T, 128)
    m = (labr[:, :, :, None] == lab[:, None, None, :])        # (B, T, 128, L)
    mk = np.ascontiguousarray(m.transpose(0, 2, 1, 3)).astype(BF16)

    in_maps = []
    for c in range(N_CORES):
        sl = slice(c * VC, (c + 1) * VC)
        in_maps.append({
            "qf": np.ascontiguousarray(qT[:, sl]),
            "sf": sT,
            "vp": np.ascontiguousarray(vp[:, sl]),
            "mk": mk,
        })
    return in_maps


def kernel(query, key, value, label_arr):
    nc = _get_bass()
    in_maps = _prepare_inputs(query, key, value, label_arr)
    res = run_bass_kernel_spmd(nc, in_maps, core_ids=list(range(N_CORES)))
    full = np.empty((B, L, V, D), dtype=F32)
    for c in range(N_CORES):
        full[:, :, c * VC:(c + 1) * VC, :] = res.results[c]["out"]
    return full



# revision 11
# speedup vs baseline: 1.5151x; 1.5151x over previous
"""Clustered attention Trainium2 kernel (8-core SPMD, sharded along v).

Math (per batch b):
    sum_tot = key.sum(axis=2)                          # (L, D)
    S[i,k,j] = query[i,k,:] . sum_tot[j,:]
    A = softmax_j(scale * S  masked to label[i]==label[j])
    out[i,k,:] = sum_j A[i,k,j] * value[j,k,:]

v2 strategy: the label mask makes attention block-diagonal after sorting
positions by label.  The host sorts rows into cluster order; the device
computes, per (batch, cluster) with Lc rows:
    S^T tile [j(part) <= Lc, (i, v)] = sum_tot_c^T @ q_c      (bf16 matmul)
    A' = exp(scale * S^T)                                      (one Act instr)
    num^T [d(part)=128, v, i]  = value_c^T @ A'_c              (per v)
    den  [i(global part), v]   = A'_c^T @ ones                 (per v, N=1)
A' needs no mask multiply at all.  num/den return to host in bf16 and the
host performs the final divide + inverse permutation.  Compute drops ~8x
versus dense masked attention (only within-cluster pairs are computed) and
HBM traffic is bf16 end-to-end: q 2MiB + v 2MiB + s 0.25MiB + num 2MiB.
"""

import numpy as np
import ml_dtypes

import concourse.bass as bass
import concourse.tile as tile
from concourse import mybir
from concourse.bass_utils import run_bass_kernel_spmd

BF16 = ml_dtypes.bfloat16
F32 = np.float32

# Problem shape (hardcoded per contract: kernel.py is self-contained).
B, L, V, D = 2, 512, 64, 128
N_CORES = 8
VC = V // N_CORES          # v slots per core
T = L // 128               # 128-row tiles along L
NCL = 8                    # number of clusters
SCALE = 1.0 / float(np.sqrt(D))


# walrus's sync-wait lowering only tolerates 1 wait per instruction; Tile can
# emit more. Hoist the excess onto preceding same-engine NoOps (the engine
# sequencer performs waits in order, so semantics are unchanged).
_WAIT_EXEMPT = {
    "InstEventSemaphore", "InstNoOp", "InstCall", "InstISA",
    "InstUnconditionalBranch", "InstCompareAndBranch", "InstRegisterMove",
    "InstBranchHint", "InstHalt",
}


def _split_waits(nc, dma_cap=1, compute_cap=1):
    fn = nc.m.functions[0]
    for blk in fn.blocks:
        il = blk.instructions
        new = []
        changed = False
        for inst in il:
            tname = type(inst).__name__
            si = inst.sync_info
            if si is not None and tname not in _WAIT_EXEMPT:
                cap = dma_cap if tname in ("InstDMACopy", "InstDMA") else compute_cap
                waits = list(si.on_wait)
                if len(waits) > cap:
                    excess, keep = waits[:-cap], waits[-cap:]
                    for w in excess:
                        nop = mybir.InstNoOp(
                            name=nc.get_next_instruction_name(),
                            sync_info=mybir.SyncInfo(on_wait=[w], on_update=[]),
                            engine=inst.engine,
                            bass_nofuse=True,
                        )
                        new.append(nop)
                    inst.sync_info = mybir.SyncInfo(
                        on_wait=keep, on_update=list(si.on_update)
                    )
                    changed = True
            new.append(inst)
        if changed:
            blk.instructions = new


def _build_bass(counts, offs):
    nc = bass.Bass()
    bf = mybir.dt.bfloat16
    f32 = mybir.dt.float32

    qs = nc.dram_tensor("qs", (B, D, L, VC), bf, kind="ExternalInput")
    ss = nc.dram_tensor("ss", (B, D, L), bf, kind="ExternalInput")
    vs = nc.dram_tensor("vs", (B, L, VC, D), bf, kind="ExternalInput")
    num = nc.dram_tensor("num", (B, D, L, VC), bf, kind="ExternalOutput")
    den = nc.dram_tensor("den", (128, B, NCL, VC), bf, kind="ExternalOutput")

    with tile.TileContext(nc) as tc:
        with (
            tc.tile_pool(name="consts", bufs=1) as cpool,
            tc.tile_pool(name="at", bufs=2) as apool,
            tc.tile_pool(name="vt", bufs=3) as vpool,
            tc.tile_pool(name="sps", bufs=2, space="PSUM") as spool,
            tc.tile_pool(name="ops", bufs=2, space="PSUM") as opool,
        ):
            qb = cpool.tile([128, B, L, VC], bf)
            sb = cpool.tile([128, B, L], bf)
            og = cpool.tile([128, B, L, VC], bf)
            dn = cpool.tile([128, B, NCL, VC], bf)
            ones = cpool.tile([128, 1], bf)
            nc.vector.memset(ones, 1.0)

            for b in range(B):
                nc.sync.dma_start(out=sb[:, b, :], in_=ss[b])
                nc.sync.dma_start(out=qb[:, b, :, :], in_=qs[b])

            evac_rot = 0
            for b in range(B):
                for c in range(NCL):
                    n = counts[b][c]
                    if n == 0:
                        continue
                    off = offs[b][c]
                    h = (n + 1) // 2

                    # value rows of this cluster, j-local on partitions.
                    vt = vpool.tile([128, VC, D], bf)
                    nc.scalar.dma_start(out=vt[0:n, :, :], in_=vs[b, off:off + n])

                    # S^T: two matmuls (one psum bank each), j on partitions.
                    sp = spool.tile([128, 2, 512], f32)
                    lhs_s = sb[:, b, off:off + n]
                    nc.tensor.matmul(
                        sp[0:n, 0, 0:h * VC], lhs_s,
                        qb[:, b, off:off + h, :], start=True, stop=True,
                    )
                    if n > h:
                        nc.tensor.matmul(
                            sp[0:n, 1, 0:(n - h) * VC], lhs_s,
                            qb[:, b, off + h:off + n, :], start=True, stop=True,
                        )

                    # A' = exp(scale * S^T) in one activation (reads up to
                    # VC garbage psum cols when n is odd; they land in at
                    # cols [n, 2h) which nothing reads).
                    at = apool.tile([128, 128, VC], bf)
                    nc.scalar.activation(
                        at[0:n, 0:2 * h, :].rearrange(
                            "p (two i) v -> p two i v", two=2),
                        sp[0:n, 0:2, 0:h * VC].rearrange(
                            "p two (i v) -> p two i v", v=VC),
                        mybir.ActivationFunctionType.Exp, scale=SCALE,
                    )

                    # num^T [d, v, i] and den [i, v] per v slot.  den reuses
                    # free columns of the sp tile (disjoint from exp's read
                    # range, within one bank — requires n <= 126).
                    po = opool.tile([128, VC, 128], f32)
                    if h * VC + VC <= 512:
                        dslot = sp[0:128, 0, h * VC:h * VC + VC]
                    else:
                        dslot = sp[0:128, 1, (n - h) * VC:(n - h) * VC + VC]
                    for v in range(VC):
                        nc.tensor.matmul(
                            po[:, v, 0:n], vt[0:n, v, :], at[0:n, 0:n, v],
                            start=True, stop=True,
                        )
                        nc.tensor.matmul(
                            dslot[0:n, v:v + 1], at[0:n, 0:n, v], ones[0:n, :],
                            start=True, stop=True,
                        )

                    # Evacuate num^T to bf16 SBUF (GPSIMD cannot read PSUM,
                    # so this is DVE work) and den into the staging tile
                    # (one DMA at the very end).
                    dst = og[:, b, off:off + n, :].rearrange("p i v -> p v i")
                    nc.vector.tensor_copy(dst, po[:, :, 0:n])
                    evac_rot += 1
                    nc.vector.tensor_copy(dn[0:n, b, c, :], dslot[0:n, :])

                half = offs[b][NCL // 2]
                nc.sync.dma_start(out=num[b, :, 0:half, :], in_=og[:, b, 0:half, :])
                nc.sync.dma_start(out=num[b, :, half:L, :], in_=og[:, b, half:L, :])
            nc.sync.dma_start(out=den[:, :, :, :], in_=dn)
    _split_waits(nc)
    return nc


_BASS_CACHE = {}


def _get_bass(counts, offs):
    key = tuple(tuple(cb) for cb in counts)
    if key not in _BASS_CACHE:
        _BASS_CACHE[key] = _build_bass(counts, offs)
    return _BASS_CACHE[key]


def _prepare(query, key, value, label_arr):
    query = np.asarray(query, dtype=F32)
    key = np.asarray(key, dtype=F32)
    value = np.asarray(value, dtype=F32)
    lab = np.asarray(label_arr)

    perms, counts, offs = [], [], []
    for b in range(B):
        cnt = np.bincount(lab[b], minlength=NCL).astype(int)
        perms.append(np.argsort(lab[b], kind="stable"))
        counts.append(cnt.tolist())
        offs.append(np.concatenate([[0], np.cumsum(cnt)]).astype(int).tolist())

    sum_tot = key.sum(axis=2)                                   # (B, L, D)

    # Sorted, transposed, bf16 packings.
    qsrt = np.empty((B, D, L, V), dtype=BF16)
    ssrt = np.empty((B, D, L), dtype=BF16)
    vsrt = np.empty((B, L, V, D), dtype=BF16)
    for b in range(B):
        p = perms[b]
        qsrt[b] = query[b, p].transpose(2, 0, 1).astype(BF16)   # (D, L, V)
        ssrt[b] = sum_tot[b, p].T.astype(BF16)                  # (D, L)
        vsrt[b] = value[b, p].astype(BF16)                      # (L, V, D)

    in_maps = []
    for cix in range(N_CORES):
        sl = slice(cix * VC, (cix + 1) * VC)
        in_maps.append({
            "qs": np.ascontiguousarray(qsrt[:, :, :, sl]),
            "ss": ssrt,
            "vs": np.ascontiguousarray(vsrt[:, :, sl, :]),
        })
    return in_maps, perms, counts, offs


def kernel(query, key, value, label_arr):
    in_maps, perms, counts, offs = _prepare(query, key, value, label_arr)
    if max(max(cb) for cb in counts) > 126:
        return _kernel_dense(query, key, value, label_arr)
    nc = _get_bass(counts, offs)
    res = run_bass_kernel_spmd(nc, in_maps, core_ids=list(range(N_CORES)))

    full = np.empty((B, L, V, D), dtype=F32)
    inv = [np.argsort(p) for p in perms]
    for cix in range(N_CORES):
        numb = np.asarray(res.results[cix]["num"], dtype=F32)   # (B, D, L, VC)
        denb = np.asarray(res.results[cix]["den"], dtype=F32)   # (128, B, NCL, VC)
        for b in range(B):
            ns = numb[b].transpose(1, 2, 0)                     # (L, VC, D)
            ds = np.empty((L, VC), dtype=F32)
            for c in range(NCL):
                off, n = offs[b][c], counts[b][c]
                ds[off:off + n] = denb[0:n, b, c, :]
            outs = ns / ds[:, :, None]
            full[b, :, cix * VC:(cix + 1) * VC, :] = outs[inv[b]]
    return full


# ---------------------------------------------------------------------------
# Dense fallback (original kernel) for degenerate label distributions where a
# cluster exceeds 128 rows.
# ---------------------------------------------------------------------------

from concourse.bass import ts as _ts


def _build_dense():
    nc = bass.Bass()
    bf = mybir.dt.bfloat16
    f32 = mybir.dt.float32
    f32r = mybir.dt.float32r

    qf = nc.dram_tensor("qf", (B, VC, D, L), f32r, kind="ExternalInput")
    sf = nc.dram_tensor("sf", (B, D, L), f32r, kind="ExternalInput")
    vp = nc.dram_tensor("vp", (B, VC, 128, T, D + 1), bf, kind="ExternalInput")
    mk = nc.dram_tensor("mk", (B, 128, T, L), bf, kind="ExternalInput")
    out = nc.dram_tensor("out", (B, L, VC, D), f32, kind="ExternalOutput")

    with tile.TileContext(nc) as tc:
        with (
            tc.tile_pool(name="consts", bufs=1) as cpool,
            tc.tile_pool(name="qin", bufs=6) as qpool,
            tc.tile_pool(name="vin", bufs=6) as vpool,
            tc.tile_pool(name="aw", bufs=4) as apool,
            tc.tile_pool(name="og", bufs=6) as opool,
            tc.tile_pool(name="rc", bufs=8) as rpool,
            tc.tile_pool(name="spsum", bufs=2, space="PSUM") as spsum,
            tc.tile_pool(name="opsum", bufs=4, space="PSUM") as opsum,
        ):
            sf_all = cpool.tile([128, B, L], f32r)
            mk_all = cpool.tile([128, B, T, L], bf)
            nc.sync.dma_start(out=sf_all, in_=sf[:, :, :].rearrange("b d l -> d b l"))
            nc.sync.dma_start(out=mk_all, in_=mk[:, :, :, :].rearrange("b p t l -> p b t l"))
            for b in range(B):
                sfb = sf_all[:, b, :]
                mkb = mk_all[:, b, :, :]
                for v in range(VC):
                    qt = qpool.tile([128, L], f32r)
                    nc.sync.dma_start(out=qt, in_=qf[b, v])
                    vv = vpool.tile([128, T, D + 1], bf)
                    nc.sync.dma_start(out=vv, in_=vp[b, v])

                    at = apool.tile([128, T, L], bf)
                    for g in range(T // 2):
                        ps = spsum.tile([128, 2, L], f32)
                        for hh in range(2):
                            jt = 2 * g + hh
                            nc.tensor.matmul(
                                ps[:, hh, :], sfb[:, _ts(jt, 128)], qt,
                                start=True, stop=True,
                            )
                        nc.scalar.activation(
                            at[:, 2 * g:2 * g + 2, :], ps,
                            mybir.ActivationFunctionType.Exp, scale=SCALE,
                        )
                        nc.vector.tensor_mul(
                            at[:, 2 * g:2 * g + 2, :],
                            at[:, 2 * g:2 * g + 2, :],
                            mkb[:, 2 * g:2 * g + 2, :],
                        )

                    og = opool.tile([128, T, D], f32)
                    for it in range(T):
                        ops = opsum.tile([128, D + 1], f32)
                        for jt in range(T):
                            nc.tensor.matmul(
                                ops, at[:, jt, _ts(it, 128)], vv[:, jt, :],
                                start=(jt == 0), stop=(jt == T - 1),
                            )
                        rc = rpool.tile([128, 1], f32)
                        nc.vector.reciprocal(rc, ops[:, D:D + 1])
                        nc.vector.tensor_scalar_mul(og[:, it, :], ops[:, 0:D], rc)
                    nc.sync.dma_start(
                        out=out[b, :, v, :].rearrange("(t p) d -> p t d", p=128),
                        in_=og,
                    )
    _split_waits(nc)
    return nc


_DENSE_CACHE = None


def _kernel_dense(query, key, value, label_arr):
    global _DENSE_CACHE
    if _DENSE_CACHE is None:
        _DENSE_CACHE = _build_dense()
    nc = _DENSE_CACHE

    query = np.asarray(query, dtype=F32)
    key = np.asarray(key, dtype=F32)
    value = np.asarray(value, dtype=F32)
    lab = np.asarray(label_arr)

    sum_tot = key.sum(axis=2)
    sT = np.ascontiguousarray(sum_tot.transpose(0, 2, 1))
    qT = np.ascontiguousarray(query.transpose(0, 2, 3, 1))

    v4 = value.reshape(B, T, 128, V, D).transpose(0, 3, 2, 1, 4)
    vp = np.empty((B, V, 128, T, D + 1), dtype=BF16)
    vp[..., :D] = v4.astype(BF16)
    vp[..., D] = np.ones((), dtype=BF16)

    labr = lab.reshape(B, T, 128)
    m = (labr[:, :, :, None] == lab[:, None, None, :])
    mkm = np.ascontiguousarray(m.transpose(0, 2, 1, 3)).astype(BF16)

    in_maps = []
    for c in range(N_CORES):
        sl = slice(c * VC, (c + 1) * VC)
        in_maps.append({
            "qf": np.ascontiguousarray(qT[:, sl]),
            "sf": sT,
            "vp": np.ascontiguousarray(vp[:, sl]),
            "mk": mkm,
        })
    res = run_bass_kernel_spmd(nc, in_maps, core_ids=list(range(N_CORES)))
    full = np.empty((B, L, V, D), dtype=F32)
    for c in range(N_CORES):
        full[:, :, c * VC:(c + 1) * VC, :] = res.results[c]["out"]
    return full


# revision 14
# speedup vs baseline: 1.6181x; 1.0680x over previous
"""Clustered attention Trainium2 kernel (8-core SPMD, sharded along v).

Math (per batch b):
    sum_tot = key.sum(axis=2)                          # (L, D)
    S[i,k,j] = query[i,k,:] . sum_tot[j,:]
    A = softmax_j(scale * S  masked to label[i]==label[j])
    out[i,k,:] = sum_j A[i,k,j] * value[j,k,:]

v2 strategy: the label mask makes attention block-diagonal after sorting
positions by label.  The host sorts rows into cluster order; the device
computes, per (batch, cluster) with Lc rows:
    S^T tile [j(part) <= Lc, (i, v)] = sum_tot_c^T @ q_c      (bf16 matmul)
    A' = exp(scale * S^T)                                      (one Act instr)
    num^T [d(part)=128, v, i]  = value_c^T @ A'_c              (per v)
    den  [i(global part), v]   = A'_c^T @ ones                 (per v, N=1)
A' needs no mask multiply at all.  num/den return to host in bf16 and the
host performs the final divide + inverse permutation.  Compute drops ~8x
versus dense masked attention (only within-cluster pairs are computed) and
HBM traffic is bf16 end-to-end: q 2MiB + v 2MiB + s 0.25MiB + num 2MiB.
"""

import numpy as np
import ml_dtypes

import concourse.bass as bass
import concourse.tile as tile
from concourse import mybir
from concourse.bass_utils import run_bass_kernel_spmd

BF16 = ml_dtypes.bfloat16
F32 = np.float32

# Problem shape (hardcoded per contract: kernel.py is self-contained).
B, L, V, D = 2, 512, 64, 128
N_CORES = 8
VC = V // N_CORES          # v slots per core
T = L // 128               # 128-row tiles along L
NCL = 8                    # number of clusters
SCALE = 1.0 / float(np.sqrt(D))


# walrus's sync-wait lowering only tolerates 1 wait per instruction; Tile can
# emit more. Hoist the excess onto preceding same-engine NoOps (the engine
# sequencer performs waits in order, so semantics are unchanged).
_WAIT_EXEMPT = {
    "InstEventSemaphore", "InstNoOp", "InstCall", "InstISA",
    "InstUnconditionalBranch", "InstCompareAndBranch", "InstRegisterMove",
    "InstBranchHint", "InstHalt",
}


def _split_waits(nc, dma_cap=1, compute_cap=1):
    fn = nc.m.functions[0]
    for blk in fn.blocks:
        il = blk.instructions
        new = []
        changed = False
        for inst in il:
            tname = type(inst).__name__
            si = inst.sync_info
            if si is not None and tname not in _WAIT_EXEMPT:
                cap = dma_cap if tname in ("InstDMACopy", "InstDMA") else compute_cap
                waits = list(si.on_wait)
                if len(waits) > cap:
                    excess, keep = waits[:-cap], waits[-cap:]
                    for w in excess:
                        nop = mybir.InstNoOp(
                            name=nc.get_next_instruction_name(),
                            sync_info=mybir.SyncInfo(on_wait=[w], on_update=[]),
                            engine=inst.engine,
                            bass_nofuse=True,
                        )
                        new.append(nop)
                    inst.sync_info = mybir.SyncInfo(
                        on_wait=keep, on_update=list(si.on_update)
                    )
                    changed = True
            new.append(inst)
        if changed:
            blk.instructions = new


def _build_bass(counts, offs):
    nc = bass.Bass()
    bf = mybir.dt.bfloat16
    f32 = mybir.dt.float32

    qs = nc.dram_tensor("qs", (B, D, L, VC), bf, kind="ExternalInput")
    ss = nc.dram_tensor("ss", (B, D, L), bf, kind="ExternalInput")
    vs = nc.dram_tensor("vs", (B, L, VC, D), bf, kind="ExternalInput")
    num = nc.dram_tensor("num", (B, D, L, VC), bf, kind="ExternalOutput")
    den = nc.dram_tensor("den", (128, B, NCL, VC), bf, kind="ExternalOutput")

    units = [(b, c) for b in range(B) for c in range(NCL) if counts[b][c] > 0]

    with tile.TileContext(nc) as tc:
        with (
            tc.tile_pool(name="consts", bufs=1) as cpool,
            tc.tile_pool(name="at", bufs=3) as apool,
            tc.tile_pool(name="vt", bufs=len(units)) as vpool,
            tc.tile_pool(name="sps", bufs=2, space="PSUM") as spool,
            tc.tile_pool(name="ops", bufs=2, space="PSUM") as opool,
        ):
            qb = cpool.tile([128, B, L, VC], bf)
            sb = cpool.tile([128, B, L], bf)
            og = cpool.tile([128, B, L, VC], bf)
            dn = cpool.tile([128, B, NCL, VC], bf)
            ones = cpool.tile([128, 1], bf)
            nc.vector.memset(ones, 1.0)

            # Input loads: s first (small), then q in per-b halves so the
            # first matmul can start after ~1/4 of the q traffic.
            nc.sync.dma_start(out=sb[:, 0, :], in_=ss[0])
            for b in range(B):
                nc.sync.dma_start(out=qb[:, b, 0:L // 2, :],
                                  in_=qs[b, :, 0:L // 2, :])
                nc.sync.dma_start(out=qb[:, b, L // 2:L, :],
                                  in_=qs[b, :, L // 2:L, :])
                if b + 1 < B:
                    nc.sync.dma_start(out=sb[:, b + 1, :], in_=ss[b + 1])

            # Value loads ride the Act queue, prefetched 2 units ahead so a
            # pending DMA never parks in front of a ready exp on the Act
            # sequencer (in-order SEQ).
            vts = {}

            def load_vt(k):
                b, c = units[k]
                n, off = counts[b][c], offs[b][c]
                vt = vpool.tile([128, VC, D], bf)
                nc.scalar.dma_start(out=vt[0:n, :, :], in_=vs[b, off:off + n])
                vts[k] = vt

            load_vt(0)
            if len(units) > 1:
                load_vt(1)

            for k, (b, c) in enumerate(units):
                n = counts[b][c]
                off = offs[b][c]
                h = (n + 1) // 2
                if k + 2 < len(units):
                    load_vt(k + 2)
                vt = vts[k]

                # S^T: two matmuls (one psum bank each), j on partitions.
                sp = spool.tile([128, 2, 512], f32)
                lhs_s = sb[:, b, off:off + n]
                nc.tensor.matmul(
                    sp[0:n, 0, 0:h * VC], lhs_s,
                    qb[:, b, off:off + h, :], start=True, stop=True,
                )
                if n > h:
                    nc.tensor.matmul(
                        sp[0:n, 1, 0:(n - h) * VC], lhs_s,
                        qb[:, b, off + h:off + n, :], start=True, stop=True,
                    )

                # A' = exp(scale * S^T) in one activation (reads up to
                # VC garbage psum cols when n is odd; they land in at
                # cols [n, 2h) which nothing reads).
                at = apool.tile([128, 128, VC], bf)
                nc.scalar.activation(
                    at[0:n, 0:2 * h, :].rearrange(
                        "p (two i) v -> p two i v", two=2),
                    sp[0:n, 0:2, 0:h * VC].rearrange(
                        "p two (i v) -> p two i v", v=VC),
                    mybir.ActivationFunctionType.Exp, scale=SCALE,
                )

                # num^T [d, v, i] and den [i, v] per v slot.  den reuses
                # free columns of the sp tile (disjoint from exp's read
                # range, within one bank — requires n <= 126).
                po = opool.tile([128, VC, 128], f32)
                if h * VC + VC <= 512:
                    dslot = sp[0:128, 0, h * VC:h * VC + VC]
                else:
                    dslot = sp[0:128, 1, (n - h) * VC:(n - h) * VC + VC]
                for v in range(VC):
                    nc.tensor.matmul(
                        po[:, v, 0:n], vt[0:n, v, :], at[0:n, 0:n, v],
                        start=True, stop=True,
                    )
                    nc.tensor.matmul(
                        dslot[0:n, v:v + 1], at[0:n, 0:n, v], ones[0:n, :],
                        start=True, stop=True,
                    )

                # Evacuate num^T to bf16 SBUF (GPSIMD cannot read PSUM, so
                # this is DVE work) and den into the staging tile.
                dst = og[:, b, off:off + n, :].rearrange("p i v -> p v i")
                nc.vector.tensor_copy(dst, po[:, :, 0:n])
                nc.vector.tensor_copy(dn[0:n, b, c, :], dslot[0:n, :])

                # Stores: num flows out in quarter-L chunks as clusters
                # complete; den per batch right after its last cluster.
                if k + 1 == len(units) or units[k + 1][0] != b:
                    q0 = 0
                    for qc in range(4):
                        q1 = offs[b][2 * (qc + 1)] if qc < 3 else L
                        nc.sync.dma_start(out=num[b, :, q0:q1, :],
                                          in_=og[:, b, q0:q1, :])
                        q0 = q1
                    nc.sync.dma_start(out=den[:, b, :, :], in_=dn[:, b])
    _split_waits(nc)
    return nc


_BASS_CACHE = {}


def _get_bass(counts, offs):
    key = tuple(tuple(cb) for cb in counts)
    if key not in _BASS_CACHE:
        _BASS_CACHE[key] = _build_bass(counts, offs)
    return _BASS_CACHE[key]


def _prepare(query, key, value, label_arr):
    query = np.asarray(query, dtype=F32)
    key = np.asarray(key, dtype=F32)
    value = np.asarray(value, dtype=F32)
    lab = np.asarray(label_arr)

    perms, counts, offs = [], [], []
    for b in range(B):
        cnt = np.bincount(lab[b], minlength=NCL).astype(int)
        perms.append(np.argsort(lab[b], kind="stable"))
        counts.append(cnt.tolist())
        offs.append(np.concatenate([[0], np.cumsum(cnt)]).astype(int).tolist())

    sum_tot = key.sum(axis=2)                                   # (B, L, D)

    # Sorted, transposed, bf16 packings.
    qsrt = np.empty((B, D, L, V), dtype=BF16)
    ssrt = np.empty((B, D, L), dtype=BF16)
    vsrt = np.empty((B, L, V, D), dtype=BF16)
    for b in range(B):
        p = perms[b]
        qsrt[b] = query[b, p].transpose(2, 0, 1).astype(BF16)   # (D, L, V)
        ssrt[b] = sum_tot[b, p].T.astype(BF16)                  # (D, L)
        vsrt[b] = value[b, p].astype(BF16)                      # (L, V, D)

    in_maps = []
    for cix in range(N_CORES):
        sl = slice(cix * VC, (cix + 1) * VC)
        in_maps.append({
            "qs": np.ascontiguousarray(qsrt[:, :, :, sl]),
            "ss": ssrt,
            "vs": np.ascontiguousarray(vsrt[:, :, sl, :]),
        })
    return in_maps, perms, counts, offs


def kernel(query, key, value, label_arr):
    in_maps, perms, counts, offs = _prepare(query, key, value, label_arr)
    if max(max(cb) for cb in counts) > 126:
        return _kernel_dense(query, key, value, label_arr)
    nc = _get_bass(counts, offs)
    res = run_bass_kernel_spmd(nc, in_maps, core_ids=list(range(N_CORES)))

    full = np.empty((B, L, V, D), dtype=F32)
    inv = [np.argsort(p) for p in perms]
    for cix in range(N_CORES):
        numb = np.asarray(res.results[cix]["num"], dtype=F32)   # (B, D, L, VC)
        denb = np.asarray(res.results[cix]["den"], dtype=F32)   # (128, B, NCL, VC)
        for b in range(B):
            ns = numb[b].transpose(1, 2, 0)                     # (L, VC, D)
            ds = np.empty((L, VC), dtype=F32)
            for c in range(NCL):
                off, n = offs[b][c], counts[b][c]
                ds[off:off + n] = denb[0:n, b, c, :]
            outs = ns / ds[:, :, None]
            full[b, :, cix * VC:(cix + 1) * VC, :] = outs[inv[b]]
    return full


# ---------------------------------------------------------------------------
# Dense fallback (original kernel) for degenerate label distributions where a
# cluster exceeds 128 rows.
# ---------------------------------------------------------------------------

from concourse.bass import ts as _ts


def _build_dense():
    nc = bass.Bass()
    bf = mybir.dt.bfloat16
    f32 = mybir.dt.float32
    f32r = mybir.dt.float32r

    qf = nc.dram_tensor("qf", (B, VC, D, L), f32r, kind="ExternalInput")
    sf = nc.dram_tensor("sf", (B, D, L), f32r, kind="ExternalInput")
    vp = nc.dram_tensor("vp", (B, VC, 128, T, D + 1), bf, kind="ExternalInput")
    mk = nc.dram_tensor("mk", (B, 128, T, L), bf, kind="ExternalInput")
    out = nc.dram_tensor("out", (B, L, VC, D), f32, kind="ExternalOutput")

    with tile.TileContext(nc) as tc:
        with (
            tc.tile_pool(name="consts", bufs=1) as cpool,
            tc.tile_pool(name="qin", bufs=6) as qpool,
            tc.tile_pool(name="vin", bufs=6) as vpool,
            tc.tile_pool(name="aw", bufs=4) as apool,
            tc.tile_pool(name="og", bufs=6) as opool,
            tc.tile_pool(name="rc", bufs=8) as rpool,
            tc.tile_pool(name="spsum", bufs=2, space="PSUM") as spsum,
            tc.tile_pool(name="opsum", bufs=4, space="PSUM") as opsum,
        ):
            sf_all = cpool.tile([128, B, L], f32r)
            mk_all = cpool.tile([128, B, T, L], bf)
            nc.sync.dma_start(out=sf_all, in_=sf[:, :, :].rearrange("b d l -> d b l"))
            nc.sync.dma_start(out=mk_all, in_=mk[:, :, :, :].rearrange("b p t l -> p b t l"))
            for b in range(B):
                sfb = sf_all[:, b, :]
                mkb = mk_all[:, b, :, :]
                for v in range(VC):
                    qt = qpool.tile([128, L], f32r)
                    nc.sync.dma_start(out=qt, in_=qf[b, v])
                    vv = vpool.tile([128, T, D + 1], bf)
                    nc.sync.dma_start(out=vv, in_=vp[b, v])

                    at = apool.tile([128, T, L], bf)
                    for g in range(T // 2):
                        ps = spsum.tile([128, 2, L], f32)
                        for hh in range(2):
                            jt = 2 * g + hh
                            nc.tensor.matmul(
                                ps[:, hh, :], sfb[:, _ts(jt, 128)], qt,
                                start=True, stop=True,
                            )
                        nc.scalar.activation(
                            at[:, 2 * g:2 * g + 2, :], ps,
                            mybir.ActivationFunctionType.Exp, scale=SCALE,
                        )
                        nc.vector.tensor_mul(
                            at[:, 2 * g:2 * g + 2, :],
                            at[:, 2 * g:2 * g + 2, :],
                            mkb[:, 2 * g:2 * g + 2, :],
                        )

                    og = opool.tile([128, T, D], f32)
                    for it in range(T):
                        ops = opsum.tile([128, D + 1], f32)
                        for jt in range(T):
                            nc.tensor.matmul(
                                ops, at[:, jt, _ts(it, 128)], vv[:, jt, :],
                                start=(jt == 0), stop=(jt == T - 1),
                            )
                        rc = rpool.tile([128, 1], f32)
                        nc.vector.reciprocal(rc, ops[:, D:D + 1])
                        nc.vector.tensor_scalar_mul(og[:, it, :], ops[:, 0:D], rc)
                    nc.sync.dma_start(
                        out=out[b, :, v, :].rearrange("(t p) d -> p t d", p=128),
                        in_=og,
                    )
    _split_waits(nc)
    return nc


_DENSE_CACHE = None


def _kernel_dense(query, key, value, label_arr):
    global _DENSE_CACHE
    if _DENSE_CACHE is None:
        _DENSE_CACHE = _build_dense()
    nc = _DENSE_CACHE

    query = np.asarray(query, dtype=F32)
    key = np.asarray(key, dtype=F32)
    value = np.asarray(value, dtype=F32)
    lab = np.asarray(label_arr)

    sum_tot = key.sum(axis=2)
    sT = np.ascontiguousarray(sum_tot.transpose(0, 2, 1))
    qT = np.ascontiguousarray(query.transpose(0, 2, 3, 1))

    v4 = value.reshape(B, T, 128, V, D).transpose(0, 3, 2, 1, 4)
    vp = np.empty((B, V, 128, T, D + 1), dtype=BF16)
    vp[..., :D] = v4.astype(BF16)
    vp[..., D] = np.ones((), dtype=BF16)

    labr = lab.reshape(B, T, 128)
    m = (labr[:, :, :, None] == lab[:, None, None, :])
    mkm = np.ascontiguousarray(m.transpose(0, 2, 1, 3)).astype(BF16)

    in_maps = []
    for c in range(N_CORES):
        sl = slice(c * VC, (c + 1) * VC)
        in_maps.append({
            "qf": np.ascontiguousarray(qT[:, sl]),
            "sf": sT,
            "vp": np.ascontiguousarray(vp[:, sl]),
            "mk": mkm,
        })
    res = run_bass_kernel_spmd(nc, in_maps, core_ids=list(range(N_CORES)))
    full = np.empty((B, L, V, D), dtype=F32)
    for c in range(N_CORES):
        full[:, :, c * VC:(c + 1) * VC, :] = res.results[c]["out"]
    return full


# revision 15
# speedup vs baseline: 1.8031x; 1.1143x over previous
"""Clustered attention Trainium2 kernel (8-core SPMD, sharded along v).

Math (per batch b):
    sum_tot = key.sum(axis=2)                          # (L, D)
    S[i,k,j] = query[i,k,:] . sum_tot[j,:]
    A = softmax_j(scale * S  masked to label[i]==label[j])
    out[i,k,:] = sum_j A[i,k,j] * value[j,k,:]

v2 strategy: the label mask makes attention block-diagonal after sorting
positions by label.  The host sorts rows into cluster order; the device
computes, per (batch, cluster) with Lc rows:
    S^T tile [j(part) <= Lc, (i, v)] = sum_tot_c^T @ q_c      (bf16 matmul)
    A' = exp(scale * S^T)                                      (one Act instr)
    num^T [d(part)=128, v, i]  = value_c^T @ A'_c              (per v)
    den  [i(global part), v]   = A'_c^T @ ones                 (per v, N=1)
A' needs no mask multiply at all.  num/den return to host in bf16 and the
host performs the final divide + inverse permutation.  Compute drops ~8x
versus dense masked attention (only within-cluster pairs are computed) and
HBM traffic is bf16 end-to-end: q 2MiB + v 2MiB + s 0.25MiB + num 2MiB.
"""

import numpy as np
import ml_dtypes

import concourse.bass as bass
import concourse.tile as tile
from concourse import mybir
from concourse.bass_utils import run_bass_kernel_spmd

BF16 = ml_dtypes.bfloat16
F32 = np.float32

# Problem shape (hardcoded per contract: kernel.py is self-contained).
B, L, V, D = 2, 512, 64, 128
N_CORES = 8
VC = V // N_CORES          # v slots per core
T = L // 128               # 128-row tiles along L
NCL = 8                    # number of clusters
SCALE = 1.0 / float(np.sqrt(D))


# walrus's sync-wait lowering only tolerates 1 wait per instruction; Tile can
# emit more. Hoist the excess onto preceding same-engine NoOps (the engine
# sequencer performs waits in order, so semantics are unchanged).
_WAIT_EXEMPT = {
    "InstEventSemaphore", "InstNoOp", "InstCall", "InstISA",
    "InstUnconditionalBranch", "InstCompareAndBranch", "InstRegisterMove",
    "InstBranchHint", "InstHalt",
}


def _split_waits(nc, dma_cap=1, compute_cap=1):
    fn = nc.m.functions[0]
    for blk in fn.blocks:
        il = blk.instructions
        new = []
        changed = False
        for inst in il:
            tname = type(inst).__name__
            si = inst.sync_info
            if si is not None and tname not in _WAIT_EXEMPT:
                cap = dma_cap if tname in ("InstDMACopy", "InstDMA") else compute_cap
                waits = list(si.on_wait)
                if len(waits) > cap:
                    excess, keep = waits[:-cap], waits[-cap:]
                    for w in excess:
                        nop = mybir.InstNoOp(
                            name=nc.get_next_instruction_name(),
                            sync_info=mybir.SyncInfo(on_wait=[w], on_update=[]),
                            engine=inst.engine,
                            bass_nofuse=True,
                        )
                        new.append(nop)
                    inst.sync_info = mybir.SyncInfo(
                        on_wait=keep, on_update=list(si.on_update)
                    )
                    changed = True
            new.append(inst)
        if changed:
            blk.instructions = new


def _build_bass(counts, offs):
    nc = bass.Bass()
    bf = mybir.dt.bfloat16
    f32 = mybir.dt.float32

    qs = nc.dram_tensor("qs", (B, D, L, VC), bf, kind="ExternalInput")
    ss = nc.dram_tensor("ss", (B, D, L), bf, kind="ExternalInput")
    vs = nc.dram_tensor("vs", (B, L, VC, D), bf, kind="ExternalInput")
    num = nc.dram_tensor("num", (B, D, L, VC), bf, kind="ExternalOutput")
    den = nc.dram_tensor("den", (128, B, NCL, VC), bf, kind="ExternalOutput")

    units = [(b, c) for b in range(B) for c in range(NCL) if counts[b][c] > 0]

    with tile.TileContext(nc) as tc:
        with (
            tc.tile_pool(name="consts", bufs=1) as cpool,
            tc.tile_pool(name="at", bufs=3) as apool,
            tc.tile_pool(name="vt", bufs=len(units)) as vpool,
            tc.tile_pool(name="sps", bufs=2, space="PSUM") as spool,
            tc.tile_pool(name="ops", bufs=2, space="PSUM") as opool,
        ):
            qb = cpool.tile([128, B, L, VC], bf)
            sb = cpool.tile([128, B, L], bf)
            og = cpool.tile([128, B, L, VC], bf)
            dn = cpool.tile([128, B, NCL, VC], bf)
            ones = cpool.tile([128, 1], bf)
            nc.vector.memset(ones, 1.0)

            # Input loads: s first (small), then q in per-b halves so the
            # first matmul can start after ~1/4 of the q traffic.
            nc.sync.dma_start(out=sb[:, 0, :], in_=ss[0])
            for b in range(B):
                nc.sync.dma_start(out=qb[:, b, 0:L // 2, :],
                                  in_=qs[b, :, 0:L // 2, :])
                nc.sync.dma_start(out=qb[:, b, L // 2:L, :],
                                  in_=qs[b, :, L // 2:L, :])
                if b + 1 < B:
                    nc.sync.dma_start(out=sb[:, b + 1, :], in_=ss[b + 1])

            # Value loads ride the Act queue, prefetched 2 units ahead so a
            # pending DMA never parks in front of a ready exp on the Act
            # sequencer (in-order SEQ).
            vts = {}

            def load_vt(k):
                b, c = units[k]
                n, off = counts[b][c], offs[b][c]
                vt = vpool.tile([128, VC, D], bf)
                nc.scalar.dma_start(out=vt[0:n, :, :], in_=vs[b, off:off + n])
                vts[k] = vt

            load_vt(0)
            if len(units) > 1:
                load_vt(1)

            for k, (b, c) in enumerate(units):
                n = counts[b][c]
                off = offs[b][c]
                h = (n + 1) // 2
                if k + 2 < len(units):
                    load_vt(k + 2)
                vt = vts[k]

                # S^T: two matmuls (one psum bank each), j on partitions.
                sp = spool.tile([128, 2, 512], f32)
                lhs_s = sb[:, b, off:off + n]
                nc.tensor.matmul(
                    sp[0:n, 0, 0:h * VC], lhs_s,
                    qb[:, b, off:off + h, :], start=True, stop=True,
                )
                if n > h:
                    nc.tensor.matmul(
                        sp[0:n, 1, 0:(n - h) * VC], lhs_s,
                        qb[:, b, off + h:off + n, :], start=True, stop=True,
                    )

                # A' = exp(scale * S^T) in one activation (reads up to
                # VC garbage psum cols when n is odd; they land in at
                # cols [n, 2h) which nothing reads).
                at = apool.tile([128, 128, VC], bf)
                nc.scalar.activation(
                    at[0:n, 0:2 * h, :].rearrange(
                        "p (two i) v -> p two i v", two=2),
                    sp[0:n, 0:2, 0:h * VC].rearrange(
                        "p two (i v) -> p two i v", v=VC),
                    mybir.ActivationFunctionType.Exp, scale=SCALE,
                )

                # num^T [d, v, i] into po cols [0, n); den [i, v] into po
                # col 127 of each v slot (n <= 126 so it never collides and
                # stays within the v slot's psum bank).  Keeping den out of
                # sp lets the sp ring recycle right after exp.
                po = opool.tile([128, VC, 128], f32)
                for v in range(VC):
                    nc.tensor.matmul(
                        po[:, v, 0:n], vt[0:n, v, :], at[0:n, 0:n, v],
                        start=True, stop=True,
                    )
                    nc.tensor.matmul(
                        po[0:n, v, 127:128], at[0:n, 0:n, v], ones[0:n, :],
                        start=True, stop=True,
                    )

                # Evacuate num^T to bf16 SBUF (GPSIMD cannot read PSUM, so
                # this is DVE work) and den into the staging tile.
                dst = og[:, b, off:off + n, :].rearrange("p i v -> p v i")
                nc.vector.tensor_copy(dst, po[:, :, 0:n])
                nc.vector.tensor_copy(dn[0:n, b, c, :], po[0:n, :, 127])

                # Stores: num flows out in quarter-L chunks as clusters
                # complete; den per batch right after its last cluster.
                if k + 1 == len(units) or units[k + 1][0] != b:
                    q0 = 0
                    for qc in range(4):
                        q1 = offs[b][2 * (qc + 1)] if qc < 3 else L
                        nc.sync.dma_start(out=num[b, :, q0:q1, :],
                                          in_=og[:, b, q0:q1, :])
                        q0 = q1
                    nc.sync.dma_start(out=den[:, b, :, :], in_=dn[:, b])
    _split_waits(nc)
    return nc


_BASS_CACHE = {}


def _get_bass(counts, offs):
    key = tuple(tuple(cb) for cb in counts)
    if key not in _BASS_CACHE:
        _BASS_CACHE[key] = _build_bass(counts, offs)
    return _BASS_CACHE[key]


def _prepare(query, key, value, label_arr):
    query = np.asarray(query, dtype=F32)
    key = np.asarray(key, dtype=F32)
    value = np.asarray(value, dtype=F32)
    lab = np.asarray(label_arr)

    perms, counts, offs = [], [], []
    for b in range(B):
        cnt = np.bincount(lab[b], minlength=NCL).astype(int)
        perms.append(np.argsort(lab[b], kind="stable"))
        counts.append(cnt.tolist())
        offs.append(np.concatenate([[0], np.cumsum(cnt)]).astype(int).tolist())

    sum_tot = key.sum(axis=2)                                   # (B, L, D)

    # Sorted, transposed, bf16 packings.
    qsrt = np.empty((B, D, L, V), dtype=BF16)
    ssrt = np.empty((B, D, L), dtype=BF16)
    vsrt = np.empty((B, L, V, D), dtype=BF16)
    for b in range(B):
        p = perms[b]
        qsrt[b] = query[b, p].transpose(2, 0, 1).astype(BF16)   # (D, L, V)
        ssrt[b] = sum_tot[b, p].T.astype(BF16)                  # (D, L)
        vsrt[b] = value[b, p].astype(BF16)                      # (L, V, D)

    in_maps = []
    for cix in range(N_CORES):
        sl = slice(cix * VC, (cix + 1) * VC)
        in_maps.append({
            "qs": np.ascontiguousarray(qsrt[:, :, :, sl]),
            "ss": ssrt,
            "vs": np.ascontiguousarray(vsrt[:, :, sl, :]),
        })
    return in_maps, perms, counts, offs


def kernel(query, key, value, label_arr):
    in_maps, perms, counts, offs = _prepare(query, key, value, label_arr)
    if max(max(cb) for cb in counts) > 126:
        return _kernel_dense(query, key, value, label_arr)
    nc = _get_bass(counts, offs)
    res = run_bass_kernel_spmd(nc, in_maps, core_ids=list(range(N_CORES)))

    full = np.empty((B, L, V, D), dtype=F32)
    inv = [np.argsort(p) for p in perms]
    for cix in range(N_CORES):
        numb = np.asarray(res.results[cix]["num"], dtype=F32)   # (B, D, L, VC)
        denb = np.asarray(res.results[cix]["den"], dtype=F32)   # (128, B, NCL, VC)
        for b in range(B):
            ns = numb[b].transpose(1, 2, 0)                     # (L, VC, D)
            ds = np.empty((L, VC), dtype=F32)
            for c in range(NCL):
                off, n = offs[b][c], counts[b][c]
                ds[off:off + n] = denb[0:n, b, c, :]
            outs = ns / ds[:, :, None]
            full[b, :, cix * VC:(cix + 1) * VC, :] = outs[inv[b]]
    return full


# ---------------------------------------------------------------------------
# Dense fallback (original kernel) for degenerate label distributions where a
# cluster exceeds 128 rows.
# ---------------------------------------------------------------------------

from concourse.bass import ts as _ts


def _build_dense():
    nc = bass.Bass()
    bf = mybir.dt.bfloat16
    f32 = mybir.dt.float32
    f32r = mybir.dt.float32r

    qf = nc.dram_tensor("qf", (B, VC, D, L), f32r, kind="ExternalInput")
    sf = nc.dram_tensor("sf", (B, D, L), f32r, kind="ExternalInput")
    vp = nc.dram_tensor("vp", (B, VC, 128, T, D + 1), bf, kind="ExternalInput")
    mk = nc.dram_tensor("mk", (B, 128, T, L), bf, kind="ExternalInput")
    out = nc.dram_tensor("out", (B, L, VC, D), f32, kind="ExternalOutput")

    with tile.TileContext(nc) as tc:
        with (
            tc.tile_pool(name="consts", bufs=1) as cpool,
            tc.tile_pool(name="qin", bufs=6) as qpool,
            tc.tile_pool(name="vin", bufs=6) as vpool,
            tc.tile_pool(name="aw", bufs=4) as apool,
            tc.tile_pool(name="og", bufs=6) as opool,
            tc.tile_pool(name="rc", bufs=8) as rpool,
            tc.tile_pool(name="spsum", bufs=2, space="PSUM") as spsum,
            tc.tile_pool(name="opsum", bufs=4, space="PSUM") as opsum,
        ):
            sf_all = cpool.tile([128, B, L], f32r)
            mk_all = cpool.tile([128, B, T, L], bf)
            nc.sync.dma_start(out=sf_all, in_=sf[:, :, :].rearrange("b d l -> d b l"))
            nc.sync.dma_start(out=mk_all, in_=mk[:, :, :, :].rearrange("b p t l -> p b t l"))
            for b in range(B):
                sfb = sf_all[:, b, :]
                mkb = mk_all[:, b, :, :]
                for v in range(VC):
                    qt = qpool.tile([128, L], f32r)
                    nc.sync.dma_start(out=qt, in_=qf[b, v])
                    vv = vpool.tile([128, T, D + 1], bf)
                    nc.sync.dma_start(out=vv, in_=vp[b, v])

                    at = apool.tile([128, T, L], bf)
                    for g in range(T // 2):
                        ps = spsum.tile([128, 2, L], f32)
                        for hh in range(2):
                            jt = 2 * g + hh
                            nc.tensor.matmul(
                                ps[:, hh, :], sfb[:, _ts(jt, 128)], qt,
                                start=True, stop=True,
                            )
                        nc.scalar.activation(
                            at[:, 2 * g:2 * g + 2, :], ps,
                            mybir.ActivationFunctionType.Exp, scale=SCALE,
                        )
                        nc.vector.tensor_mul(
                            at[:, 2 * g:2 * g + 2, :],
                            at[:, 2 * g:2 * g + 2, :],
                            mkb[:, 2 * g:2 * g + 2, :],
                        )

                    og = opool.tile([128, T, D], f32)
                    for it in range(T):
                        ops = opsum.tile([128, D + 1], f32)
                        for jt in range(T):
                            nc.tensor.matmul(
                                ops, at[:, jt, _ts(it, 128)], vv[:, jt, :],
                                start=(jt == 0), stop=(jt == T - 1),
                            )
                        rc = rpool.tile([128, 1], f32)
                        nc.vector.reciprocal(rc, ops[:, D:D + 1])
                        nc.vector.tensor_scalar_mul(og[:, it, :], ops[:, 0:D], rc)
                    nc.sync.dma_start(
                        out=out[b, :, v, :].rearrange("(t p) d -> p t d", p=128),
                        in_=og,
                    )
    _split_waits(nc)
    return nc


_DENSE_CACHE = None


def _kernel_dense(query, key, value, label_arr):
    global _DENSE_CACHE
    if _DENSE_CACHE is None:
        _DENSE_CACHE = _build_dense()
    nc = _DENSE_CACHE

    query = np.asarray(query, dtype=F32)
    key = np.asarray(key, dtype=F32)
    value = np.asarray(value, dtype=F32)
    lab = np.asarray(label_arr)

    sum_tot = key.sum(axis=2)
    sT = np.ascontiguousarray(sum_tot.transpose(0, 2, 1))
    qT = np.ascontiguousarray(query.transpose(0, 2, 3, 1))

    v4 = value.reshape(B, T, 128, V, D).transpose(0, 3, 2, 1, 4)
    vp = np.empty((B, V, 128, T, D + 1), dtype=BF16)
    vp[..., :D] = v4.astype(BF16)
    vp[..., D] = np.ones((), dtype=BF16)

    labr = lab.reshape(B, T, 128)
    m = (labr[:, :, :, None] == lab[:, None, None, :])
    mkm = np.ascontiguousarray(m.transpose(0, 2, 1, 3)).astype(BF16)

    in_maps = []
    for c in range(N_CORES):
        sl = slice(c * VC, (c + 1) * VC)
        in_maps.append({
            "qf": np.ascontiguousarray(qT[:, sl]),
            "sf": sT,
            "vp": np.ascontiguousarray(vp[:, sl]),
            "mk": mkm,
        })
    res = run_bass_kernel_spmd(nc, in_maps, core_ids=list(range(N_CORES)))
    full = np.empty((B, L, V, D), dtype=F32)
    for c in range(N_CORES):
        full[:, :, c * VC:(c + 1) * VC, :] = res.results[c]["out"]
    return full


# revision 18
# speedup vs baseline: 1.8479x; 1.0249x over previous
"""Clustered attention Trainium2 kernel (8-core SPMD, sharded along v).

Math (per batch b):
    sum_tot = key.sum(axis=2)                          # (L, D)
    S[i,k,j] = query[i,k,:] . sum_tot[j,:]
    A = softmax_j(scale * S  masked to label[i]==label[j])
    out[i,k,:] = sum_j A[i,k,j] * value[j,k,:]

v2 strategy: the label mask makes attention block-diagonal after sorting
positions by label.  The host sorts rows into cluster order; the device
computes, per (batch, cluster) with Lc rows:
    S^T tile [j(part) <= Lc, (i, v)] = sum_tot_c^T @ q_c      (bf16 matmul)
    A' = exp(scale * S^T)                                      (one Act instr)
    num^T [d(part)=128, v, i]  = value_c^T @ A'_c              (per v)
    den  [i(global part), v]   = A'_c^T @ ones                 (per v, N=1)
A' needs no mask multiply at all.  num/den return to host in bf16 and the
host performs the final divide + inverse permutation.  Compute drops ~8x
versus dense masked attention (only within-cluster pairs are computed) and
HBM traffic is bf16 end-to-end: q 2MiB + v 2MiB + s 0.25MiB + num 2MiB.
"""

import numpy as np
import ml_dtypes

import concourse.bass as bass
import concourse.tile as tile
from concourse import mybir
from concourse.bass_utils import run_bass_kernel_spmd

BF16 = ml_dtypes.bfloat16
F32 = np.float32

# Problem shape (hardcoded per contract: kernel.py is self-contained).
B, L, V, D = 2, 512, 64, 128
N_CORES = 8
VC = V // N_CORES          # v slots per core
T = L // 128               # 128-row tiles along L
NCL = 8                    # number of clusters
SCALE = 1.0 / float(np.sqrt(D))


# walrus's sync-wait lowering only tolerates 1 wait per instruction; Tile can
# emit more. Hoist the excess onto preceding same-engine NoOps (the engine
# sequencer performs waits in order, so semantics are unchanged).
_WAIT_EXEMPT = {
    "InstEventSemaphore", "InstNoOp", "InstCall", "InstISA",
    "InstUnconditionalBranch", "InstCompareAndBranch", "InstRegisterMove",
    "InstBranchHint", "InstHalt",
}


def _split_waits(nc, dma_cap=1, compute_cap=1):
    fn = nc.m.functions[0]
    for blk in fn.blocks:
        il = blk.instructions
        new = []
        changed = False
        for inst in il:
            tname = type(inst).__name__
            si = inst.sync_info
            if si is not None and tname not in _WAIT_EXEMPT:
                cap = dma_cap if tname in ("InstDMACopy", "InstDMA") else compute_cap
                waits = list(si.on_wait)
                if len(waits) > cap:
                    excess, keep = waits[:-cap], waits[-cap:]
                    for w in excess:
                        nop = mybir.InstNoOp(
                            name=nc.get_next_instruction_name(),
                            sync_info=mybir.SyncInfo(on_wait=[w], on_update=[]),
                            engine=inst.engine,
                            bass_nofuse=True,
                        )
                        new.append(nop)
                    inst.sync_info = mybir.SyncInfo(
                        on_wait=keep, on_update=list(si.on_update)
                    )
                    changed = True
            new.append(inst)
        if changed:
            blk.instructions = new


def _build_bass(counts, offs):
    nc = bass.Bass()
    bf = mybir.dt.bfloat16
    f32 = mybir.dt.float32

    qs = nc.dram_tensor("qs", (B, D, L, VC), bf, kind="ExternalInput")
    ss = nc.dram_tensor("ss", (B, D, L), bf, kind="ExternalInput")
    vs = nc.dram_tensor("vs", (B, L, VC, D), bf, kind="ExternalInput")
    num = nc.dram_tensor("num", (B, D, L, VC), bf, kind="ExternalOutput")
    den = nc.dram_tensor("den", (128, B, NCL, VC), bf, kind="ExternalOutput")

    units = [(b, c) for b in range(B) for c in range(NCL) if counts[b][c] > 0]

    with tile.TileContext(nc) as tc:
        with (
            tc.tile_pool(name="consts", bufs=1) as cpool,
            tc.tile_pool(name="at", bufs=3) as apool,
            tc.tile_pool(name="vt", bufs=len(units)) as vpool,
            tc.tile_pool(name="sps", bufs=2, space="PSUM") as spool,
            tc.tile_pool(name="ops", bufs=4, space="PSUM") as opool,
        ):
            qb = cpool.tile([128, B, L, VC], bf)
            sb = cpool.tile([128, B, L], bf)
            og = cpool.tile([128, B, L, VC], bf)
            dn = cpool.tile([128, B, NCL, VC], bf)
            ones = cpool.tile([128, 1], bf)
            nc.vector.memset(ones, 1.0)

            # Input loads: s first (small), then q in per-b halves so the
            # first matmul can start after ~1/4 of the q traffic.
            nc.sync.dma_start(out=sb[:, 0, :], in_=ss[0])
            for b in range(B):
                lo = offs[b][2]
                nc.sync.dma_start(out=qb[:, b, 0:lo, :], in_=qs[b, :, 0:lo, :])
                nc.sync.dma_start(out=qb[:, b, lo:L, :], in_=qs[b, :, lo:L, :])
                if b + 1 < B:
                    nc.sync.dma_start(out=sb[:, b + 1, :], in_=ss[b + 1])

            # Value loads ride the Act queue, prefetched 2 units ahead so a
            # pending DMA never parks in front of a ready exp on the Act
            # sequencer (in-order SEQ).
            vts = {}

            def load_vt(k):
                b, c = units[k]
                n, off = counts[b][c], offs[b][c]
                vt = vpool.tile([128, VC, D], bf)
                nc.scalar.dma_start(out=vt[0:n, :, :], in_=vs[b, off:off + n])
                vts[k] = vt

            load_vt(0)
            if len(units) > 1:
                load_vt(1)

            for k, (b, c) in enumerate(units):
                n = counts[b][c]
                off = offs[b][c]
                h = (n + 1) // 2
                if k + 2 < len(units):
                    load_vt(k + 2)
                vt = vts[k]

                # S^T: two matmuls (one psum bank each), j on partitions.
                sp = spool.tile([128, 2, 512], f32)
                lhs_s = sb[:, b, off:off + n]
                nc.tensor.matmul(
                    sp[0:n, 0, 0:h * VC], lhs_s,
                    qb[:, b, off:off + h, :], start=True, stop=True,
                )
                if n > h:
                    nc.tensor.matmul(
                        sp[0:n, 1, 0:(n - h) * VC], lhs_s,
                        qb[:, b, off + h:off + n, :], start=True, stop=True,
                    )

                # A' = exp(scale * S^T) in one activation (reads up to
                # VC garbage psum cols when n is odd; they land in at
                # cols [n, 2h) which nothing reads).
                at = apool.tile([128, 128, VC], bf)
                nc.scalar.activation(
                    at[0:n, 0:2 * h, :].rearrange(
                        "p (two i) v -> p two i v", two=2),
                    sp[0:n, 0:2, 0:h * VC].rearrange(
                        "p two (i v) -> p two i v", v=VC),
                    mybir.ActivationFunctionType.Exp, scale=SCALE,
                )

                # num^T [d, v, i] into po cols [0, n); den [i, v] into po
                # col 127 of each v slot (n <= 126 so it never collides and
                # stays within the v slot's psum bank).  po is split into two
                # 4-v half tiles (1 psum bank each) so evacuation of the
                # first half overlaps the second half's matmuls and the po
                # ring recycles at half-unit granularity.
                HV = VC // 2
                for g in range(2):
                    po = opool.tile([128, HV, 128], f32)
                    for vg in range(HV):
                        v = g * HV + vg
                        nc.tensor.matmul(
                            po[:, vg, 0:n], vt[0:n, v, :], at[0:n, 0:n, v],
                            start=True, stop=True,
                        )
                        nc.tensor.matmul(
                            po[0:n, vg, 127:128], at[0:n, 0:n, v],
                            ones[0:n, :], start=True, stop=True,
                        )
                    dst = og[:, b, off:off + n, g * HV:(g + 1) * HV]
                    nc.vector.tensor_copy(
                        dst.rearrange("p i v -> p v i"), po[:, :, 0:n])
                    nc.vector.tensor_copy(
                        dn[0:n, b, c, g * HV:(g + 1) * HV], po[0:n, :, 127])

                # Stores: num flows out in quarter-L chunks as clusters
                # complete; den per batch right after its last cluster.
                if k + 1 == len(units) or units[k + 1][0] != b:
                    q0 = 0
                    for qc in range(4):
                        q1 = offs[b][2 * (qc + 1)] if qc < 3 else L
                        nc.sync.dma_start(out=num[b, :, q0:q1, :],
                                          in_=og[:, b, q0:q1, :])
                        q0 = q1
                    nc.sync.dma_start(out=den[:, b, :, :], in_=dn[:, b])
    _split_waits(nc)
    return nc


_BASS_CACHE = {}


def _get_bass(counts, offs):
    key = tuple(tuple(cb) for cb in counts)
    if key not in _BASS_CACHE:
        _BASS_CACHE[key] = _build_bass(counts, offs)
    return _BASS_CACHE[key]


def _prepare(query, key, value, label_arr):
    query = np.asarray(query, dtype=F32)
    key = np.asarray(key, dtype=F32)
    value = np.asarray(value, dtype=F32)
    lab = np.asarray(label_arr)

    perms, counts, offs = [], [], []
    for b in range(B):
        cnt = np.bincount(lab[b], minlength=NCL).astype(int)
        perms.append(np.argsort(lab[b], kind="stable"))
        counts.append(cnt.tolist())
        offs.append(np.concatenate([[0], np.cumsum(cnt)]).astype(int).tolist())

    sum_tot = key.sum(axis=2)                                   # (B, L, D)

    # Sorted, transposed, bf16 packings.
    qsrt = np.empty((B, D, L, V), dtype=BF16)
    ssrt = np.empty((B, D, L), dtype=BF16)
    vsrt = np.empty((B, L, V, D), dtype=BF16)
    for b in range(B):
        p = perms[b]
        qsrt[b] = query[b, p].transpose(2, 0, 1).astype(BF16)   # (D, L, V)
        ssrt[b] = sum_tot[b, p].T.astype(BF16)                  # (D, L)
        vsrt[b] = value[b, p].astype(BF16)                      # (L, V, D)

    in_maps = []
    for cix in range(N_CORES):
        sl = slice(cix * VC, (cix + 1) * VC)
        in_maps.append({
            "qs": np.ascontiguousarray(qsrt[:, :, :, sl]),
            "ss": ssrt,
            "vs": np.ascontiguousarray(vsrt[:, :, sl, :]),
        })
    return in_maps, perms, counts, offs


def kernel(query, key, value, label_arr):
    in_maps, perms, counts, offs = _prepare(query, key, value, label_arr)
    if max(max(cb) for cb in counts) > 126:
        return _kernel_dense(query, key, value, label_arr)
    nc = _get_bass(counts, offs)
    res = run_bass_kernel_spmd(nc, in_maps, core_ids=list(range(N_CORES)))

    full = np.empty((B, L, V, D), dtype=F32)
    inv = [np.argsort(p) for p in perms]
    for cix in range(N_CORES):
        numb = np.asarray(res.results[cix]["num"], dtype=F32)   # (B, D, L, VC)
        denb = np.asarray(res.results[cix]["den"], dtype=F32)   # (128, B, NCL, VC)
        for b in range(B):
            ns = numb[b].transpose(1, 2, 0)                     # (L, VC, D)
            ds = np.empty((L, VC), dtype=F32)
            for c in range(NCL):
                off, n = offs[b][c], counts[b][c]
                ds[off:off + n] = denb[0:n, b, c, :]
            outs = ns / ds[:, :, None]
            full[b, :, cix * VC:(cix + 1) * VC, :] = outs[inv[b]]
    return full


# ---------------------------------------------------------------------------
# Dense fallback (original kernel) for degenerate label distributions where a
# cluster exceeds 128 rows.
# ---------------------------------------------------------------------------

from concourse.bass import ts as _ts


def _build_dense():
    nc = bass.Bass()
    bf = mybir.dt.bfloat16
    f32 = mybir.dt.float32
    f32r = mybir.dt.float32r

    qf = nc.dram_tensor("qf", (B, VC, D, L), f32r, kind="ExternalInput")
    sf = nc.dram_tensor("sf", (B, D, L), f32r, kind="ExternalInput")
    vp = nc.dram_tensor("vp", (B, VC, 128, T, D + 1), bf, kind="ExternalInput")
    mk = nc.dram_tensor("mk", (B, 128, T, L), bf, kind="ExternalInput")
    out = nc.dram_tensor("out", (B, L, VC, D), f32, kind="ExternalOutput")

    with tile.TileContext(nc) as tc:
        with (
            tc.tile_pool(name="consts", bufs=1) as cpool,
            tc.tile_pool(name="qin", bufs=6) as qpool,
            tc.tile_pool(name="vin", bufs=6) as vpool,
            tc.tile_pool(name="aw", bufs=4) as apool,
            tc.tile_pool(name="og", bufs=6) as opool,
            tc.tile_pool(name="rc", bufs=8) as rpool,
            tc.tile_pool(name="spsum", bufs=2, space="PSUM") as spsum,
            tc.tile_pool(name="opsum", bufs=4, space="PSUM") as opsum,
        ):
            sf_all = cpool.tile([128, B, L], f32r)
            mk_all = cpool.tile([128, B, T, L], bf)
            nc.sync.dma_start(out=sf_all, in_=sf[:, :, :].rearrange("b d l -> d b l"))
            nc.sync.dma_start(out=mk_all, in_=mk[:, :, :, :].rearrange("b p t l -> p b t l"))
            for b in range(B):
                sfb = sf_all[:, b, :]
                mkb = mk_all[:, b, :, :]
                for v in range(VC):
                    qt = qpool.tile([128, L], f32r)
                    nc.sync.dma_start(out=qt, in_=qf[b, v])
                    vv = vpool.tile([128, T, D + 1], bf)
                    nc.sync.dma_start(out=vv, in_=vp[b, v])

                    at = apool.tile([128, T, L], bf)
                    for g in range(T // 2):
                        ps = spsum.tile([128, 2, L], f32)
                        for hh in range(2):
                            jt = 2 * g + hh
                            nc.tensor.matmul(
                                ps[:, hh, :], sfb[:, _ts(jt, 128)], qt,
                                start=True, stop=True,
                            )
                        nc.scalar.activation(
                            at[:, 2 * g:2 * g + 2, :], ps,
                            mybir.ActivationFunctionType.Exp, scale=SCALE,
                        )
                        nc.vector.tensor_mul(
                            at[:, 2 * g:2 * g + 2, :],
                            at[:, 2 * g:2 * g + 2, :],
                            mkb[:, 2 * g:2 * g + 2, :],
                        )

                    og = opool.tile([128, T, D], f32)
                    for it in range(T):
                        ops = opsum.tile([128, D + 1], f32)
                        for jt in range(T):
                            nc.tensor.matmul(
                                ops, at[:, jt, _ts(it, 128)], vv[:, jt, :],
                                start=(jt == 0), stop=(jt == T - 1),
                            )
                        rc = rpool.tile([128, 1], f32)
                        nc.vector.reciprocal(rc, ops[:, D:D + 1])
                        nc.vector.tensor_scalar_mul(og[:, it, :], ops[:, 0:D], rc)
                    nc.sync.dma_start(
                        out=out[b, :, v, :].rearrange("(t p) d -> p t d", p=128),
                        in_=og,
                    )
    _split_waits(nc)
    return nc


_DENSE_CACHE = None


def _kernel_dense(query, key, value, label_arr):
    global _DENSE_CACHE
    if _DENSE_CACHE is None:
        _DENSE_CACHE = _build_dense()
    nc = _DENSE_CACHE

    query = np.asarray(query, dtype=F32)
    key = np.asarray(key, dtype=F32)
    value = np.asarray(value, dtype=F32)
    lab = np.asarray(label_arr)

    sum_tot = key.sum(axis=2)
    sT = np.ascontiguousarray(sum_tot.transpose(0, 2, 1))
    qT = np.ascontiguousarray(query.transpose(0, 2, 3, 1))

    v4 = value.reshape(B, T, 128, V, D).transpose(0, 3, 2, 1, 4)
    vp = np.empty((B, V, 128, T, D + 1), dtype=BF16)
    vp[..., :D] = v4.astype(BF16)
    vp[..., D] = np.ones((), dtype=BF16)

    labr = lab.reshape(B, T, 128)
    m = (labr[:, :, :, None] == lab[:, None, None, :])
    mkm = np.ascontiguousarray(m.transpose(0, 2, 1, 3)).astype(BF16)

    in_maps = []
    for c in range(N_CORES):
        sl = slice(c * VC, (c + 1) * VC)
        in_maps.append({
            "qf": np.ascontiguousarray(qT[:, sl]),
            "sf": sT,
            "vp": np.ascontiguousarray(vp[:, sl]),
            "mk": mkm,
        })
    res = run_bass_kernel_spmd(nc, in_maps, core_ids=list(range(N_CORES)))
    full = np.empty((B, L, V, D), dtype=F32)
    for c in range(N_CORES):
        full[:, :, c * VC:(c + 1) * VC, :] = res.results[c]["out"]
    return full


# revision 24
# speedup vs baseline: 2.0375x; 1.1026x over previous
"""Clustered attention Trainium2 kernel (8-core SPMD, sharded along v).

Math (per batch b):
    sum_tot = key.sum(axis=2)                          # (L, D)
    S[i,k,j] = query[i,k,:] . sum_tot[j,:]
    A = softmax_j(scale * S  masked to label[i]==label[j])
    out[i,k,:] = sum_j A[i,k,j] * value[j,k,:]

v2 strategy: the label mask makes attention block-diagonal after sorting
positions by label.  The host sorts rows into cluster order; the device
computes, per (batch, cluster) with Lc rows:
    S^T tile [j(part) <= Lc, (i, v)] = sum_tot_c^T @ q_c      (bf16 matmul)
    A' = exp(scale * S^T)                                      (one Act instr)
    num^T [d(part)=128, v, i]  = value_c^T @ A'_c              (per v)
    den  [i(global part), v]   = A'_c^T @ ones                 (per v, N=1)
A' needs no mask multiply at all.  num/den return to host in bf16 and the
host performs the final divide + inverse permutation.  Compute drops ~8x
versus dense masked attention (only within-cluster pairs are computed) and
HBM traffic is bf16 end-to-end: q 2MiB + v 2MiB + s 0.25MiB + num 2MiB.
"""

import numpy as np
import ml_dtypes

import concourse.bass as bass
import concourse.tile as tile
from concourse import mybir
from concourse.bass_utils import run_bass_kernel_spmd

BF16 = ml_dtypes.bfloat16
F32 = np.float32

# Problem shape (hardcoded per contract: kernel.py is self-contained).
B, L, V, D = 2, 512, 64, 128
N_CORES = 8
VC = V // N_CORES          # v slots per core
T = L // 128               # 128-row tiles along L
NCL = 8                    # number of clusters
SCALE = 1.0 / float(np.sqrt(D))


# walrus's sync-wait lowering only tolerates 1 wait per instruction; Tile can
# emit more. Hoist the excess onto preceding same-engine NoOps (the engine
# sequencer performs waits in order, so semantics are unchanged).
_WAIT_EXEMPT = {
    "InstEventSemaphore", "InstNoOp", "InstCall", "InstISA",
    "InstUnconditionalBranch", "InstCompareAndBranch", "InstRegisterMove",
    "InstBranchHint", "InstHalt",
}


def _split_waits(nc, dma_cap=1, compute_cap=1):
    fn = nc.m.functions[0]
    for blk in fn.blocks:
        il = blk.instructions
        new = []
        changed = False
        for inst in il:
            tname = type(inst).__name__
            si = inst.sync_info
            if si is not None and tname not in _WAIT_EXEMPT:
                cap = dma_cap if tname in ("InstDMACopy", "InstDMA") else compute_cap
                waits = list(si.on_wait)
                if len(waits) > cap:
                    excess, keep = waits[:-cap], waits[-cap:]
                    for w in excess:
                        nop = mybir.InstNoOp(
                            name=nc.get_next_instruction_name(),
                            sync_info=mybir.SyncInfo(on_wait=[w], on_update=[]),
                            engine=inst.engine,
                            bass_nofuse=True,
                        )
                        new.append(nop)
                    inst.sync_info = mybir.SyncInfo(
                        on_wait=keep, on_update=list(si.on_update)
                    )
                    changed = True
            new.append(inst)
        if changed:
            blk.instructions = new


def _build_bass(counts, offs):
    nc = bass.Bass()
    bf = mybir.dt.bfloat16
    f32 = mybir.dt.float32

    qs = nc.dram_tensor("qs", (B, D, L, VC), bf, kind="ExternalInput")
    ss = nc.dram_tensor("ss", (B, D, L), bf, kind="ExternalInput")
    vs = nc.dram_tensor("vs", (B, L, VC, D), bf, kind="ExternalInput")
    # num rows interleave an extra den row per cluster: cluster c occupies
    # rows [off_c + c, off_c + c + n_c) (numerator, d on the D axis) plus row
    # off_c + c + n_c whose D axis is indexed by i and holds the denominator.
    num = nc.dram_tensor("num", (B, D, L + NCL, VC), bf, kind="ExternalOutput")

    units = [(b, c) for b in range(B) for c in range(NCL) if counts[b][c] > 0]

    with tile.TileContext(nc) as tc:
        with (
            tc.tile_pool(name="consts", bufs=1) as cpool,
            tc.tile_pool(name="at", bufs=3) as apool,
            tc.tile_pool(name="vt", bufs=len(units)) as vpool,
            tc.tile_pool(name="sps", bufs=2, space="PSUM") as spool,
            tc.tile_pool(name="ops", bufs=2, space="PSUM") as opool,
        ):
            qb = cpool.tile([128, B, L, VC], bf)
            sb = cpool.tile([128, B, L], bf)
            og = cpool.tile([128, B, L + NCL, VC], bf)
            ones = cpool.tile([128, 1], bf)
            nc.vector.memset(ones, 1.0)

            # Input loads: s first (small), then q in per-b halves so the
            # first matmul can start after ~1/4 of the q traffic.
            nc.sync.dma_start(out=sb[:, 0, :], in_=ss[0])
            for b in range(B):
                lo = offs[b][2]
                nc.sync.dma_start(out=qb[:, b, 0:lo, :], in_=qs[b, :, 0:lo, :])
                nc.sync.dma_start(out=qb[:, b, lo:L, :], in_=qs[b, :, lo:L, :])
                if b + 1 < B:
                    nc.sync.dma_start(out=sb[:, b + 1, :], in_=ss[b + 1])

            # Value loads ride the Act queue, prefetched 2 units ahead so a
            # pending DMA never parks in front of a ready exp on the Act
            # sequencer (in-order SEQ).
            vts = {}

            def load_vt(k):
                b, c = units[k]
                n, off = counts[b][c], offs[b][c]
                vt = vpool.tile([128, VC, D], bf)
                nc.scalar.dma_start(out=vt[0:n, :, :], in_=vs[b, off:off + n])
                vts[k] = vt

            load_vt(0)
            if len(units) > 1:
                load_vt(1)

            issued = [set() for _ in range(B)]
            for k, (b, c) in enumerate(units):
                n = counts[b][c]
                off = offs[b][c]
                h = (n + 1) // 2
                if k + 2 < len(units):
                    load_vt(k + 2)
                vt = vts[k]

                # S^T: two matmuls (one psum bank each), j on partitions.
                sp = spool.tile([128, 2, 512], f32)
                lhs_s = sb[:, b, off:off + n]
                nc.tensor.matmul(
                    sp[0:n, 0, 0:h * VC], lhs_s,
                    qb[:, b, off:off + h, :], start=True, stop=True,
                )
                if n > h:
                    nc.tensor.matmul(
                        sp[0:n, 1, 0:(n - h) * VC], lhs_s,
                        qb[:, b, off + h:off + n, :], start=True, stop=True,
                    )

                # A' = exp(scale * S^T) in one activation (reads up to
                # VC garbage psum cols when n is odd; they land in at
                # cols [n, 2h) which nothing reads).
                at = apool.tile([128, 128, VC], bf)
                nc.scalar.activation(
                    at[0:n, 0:2 * h, :].rearrange(
                        "p (two i) v -> p two i v", two=2),
                    sp[0:n, 0:2, 0:h * VC].rearrange(
                        "p two (i v) -> p two i v", v=VC),
                    mybir.ActivationFunctionType.Exp, scale=SCALE,
                )

                # num^T [d, v, i] into po cols [0, n); den [i, v] into po
                # col n of each v slot (within the 512B slot since n <= 126).
                # One evacuation then carries num AND den to og in a single
                # DVE pass — den lands in og row off+c+n with i on partitions.
                po = opool.tile([128, VC, 128], f32)
                for v in range(VC):
                    nc.tensor.matmul(
                        po[:, v, 0:n], vt[0:n, v, :], at[0:n, 0:n, v],
                        start=True, stop=True,
                    )
                    nc.tensor.matmul(
                        po[0:n, v, n:n + 1], at[0:n, 0:n, v],
                        ones[0:n, :], start=True, stop=True,
                    )
                dst = og[:, b, off + c:off + c + n + 1, :]
                nc.vector.tensor_copy(
                    dst.rearrange("p i v -> p v i"), po[:, :, 0:n + 1])

                # num flows out in quarter-L chunks as cluster pairs finish.
                last_of_b = k + 1 == len(units) or units[k + 1][0] != b
                for qc in range(4):
                    if qc in issued[b]:
                        continue
                    if last_of_b or c >= 2 * qc + 1:
                        r0 = offs[b][2 * qc] + 2 * qc
                        r1 = offs[b][2 * (qc + 1)] + 2 * (qc + 1)
                        nc.sync.dma_start(out=num[b, :, r0:r1, :],
                                          in_=og[:, b, r0:r1, :])
                        issued[b].add(qc)
    _split_waits(nc)
    return nc


_BASS_CACHE = {}


def _get_bass(counts, offs):
    key = tuple(tuple(cb) for cb in counts)
    if key not in _BASS_CACHE:
        _BASS_CACHE[key] = _build_bass(counts, offs)
    return _BASS_CACHE[key]


def _prepare(query, key, value, label_arr):
    query = np.asarray(query, dtype=F32)
    key = np.asarray(key, dtype=F32)
    value = np.asarray(value, dtype=F32)
    lab = np.asarray(label_arr)

    perms, counts, offs = [], [], []
    for b in range(B):
        cnt = np.bincount(lab[b], minlength=NCL).astype(int)
        perms.append(np.argsort(lab[b], kind="stable"))
        counts.append(cnt.tolist())
        offs.append(np.concatenate([[0], np.cumsum(cnt)]).astype(int).tolist())

    sum_tot = key.sum(axis=2)                                   # (B, L, D)

    # Sorted, transposed, bf16 packings.
    qsrt = np.empty((B, D, L, V), dtype=BF16)
    ssrt = np.empty((B, D, L), dtype=BF16)
    vsrt = np.empty((B, L, V, D), dtype=BF16)
    for b in range(B):
        p = perms[b]
        qsrt[b] = query[b, p].transpose(2, 0, 1).astype(BF16)   # (D, L, V)
        ssrt[b] = sum_tot[b, p].T.astype(BF16)                  # (D, L)
        vsrt[b] = value[b, p].astype(BF16)                      # (L, V, D)

    in_maps = []
    for cix in range(N_CORES):
        sl = slice(cix * VC, (cix + 1) * VC)
        in_maps.append({
            "qs": np.ascontiguousarray(qsrt[:, :, :, sl]),
            "ss": ssrt,
            "vs": np.ascontiguousarray(vsrt[:, :, sl, :]),
        })
    return in_maps, perms, counts, offs


def kernel(query, key, value, label_arr):
    in_maps, perms, counts, offs = _prepare(query, key, value, label_arr)
    if max(max(cb) for cb in counts) > 126:
        return _kernel_dense(query, key, value, label_arr)
    nc = _get_bass(counts, offs)
    res = run_bass_kernel_spmd(nc, in_maps, core_ids=list(range(N_CORES)))

    full = np.empty((B, L, V, D), dtype=F32)
    inv = [np.argsort(p) for p in perms]
    for cix in range(N_CORES):
        numb = np.asarray(res.results[cix]["num"], dtype=F32)   # (B,D,L+NCL,VC)
        for b in range(B):
            ns = np.empty((L, VC, D), dtype=F32)
            ds = np.empty((L, VC), dtype=F32)
            for c in range(NCL):
                off, n = offs[b][c], counts[b][c]
                r = off + c
                ns[off:off + n] = numb[b, :, r:r + n, :].transpose(1, 2, 0)
                ds[off:off + n] = numb[b, 0:n, r + n, :]
            outs = ns / ds[:, :, None]
            full[b, :, cix * VC:(cix + 1) * VC, :] = outs[inv[b]]
    return full


# ---------------------------------------------------------------------------
# Dense fallback (original kernel) for degenerate label distributions where a
# cluster exceeds 128 rows.
# ---------------------------------------------------------------------------

from concourse.bass import ts as _ts


def _build_dense():
    nc = bass.Bass()
    bf = mybir.dt.bfloat16
    f32 = mybir.dt.float32
    f32r = mybir.dt.float32r

    qf = nc.dram_tensor("qf", (B, VC, D, L), f32r, kind="ExternalInput")
    sf = nc.dram_tensor("sf", (B, D, L), f32r, kind="ExternalInput")
    vp = nc.dram_tensor("vp", (B, VC, 128, T, D + 1), bf, kind="ExternalInput")
    mk = nc.dram_tensor("mk", (B, 128, T, L), bf, kind="ExternalInput")
    out = nc.dram_tensor("out", (B, L, VC, D), f32, kind="ExternalOutput")

    with tile.TileContext(nc) as tc:
        with (
            tc.tile_pool(name="consts", bufs=1) as cpool,
            tc.tile_pool(name="qin", bufs=6) as qpool,
            tc.tile_pool(name="vin", bufs=6) as vpool,
            tc.tile_pool(name="aw", bufs=4) as apool,
            tc.tile_pool(name="og", bufs=6) as opool,
            tc.tile_pool(name="rc", bufs=8) as rpool,
            tc.tile_pool(name="spsum", bufs=2, space="PSUM") as spsum,
            tc.tile_pool(name="opsum", bufs=4, space="PSUM") as opsum,
        ):
            sf_all = cpool.tile([128, B, L], f32r)
            mk_all = cpool.tile([128, B, T, L], bf)
            nc.sync.dma_start(out=sf_all, in_=sf[:, :, :].rearrange("b d l -> d b l"))
            nc.sync.dma_start(out=mk_all, in_=mk[:, :, :, :].rearrange("b p t l -> p b t l"))
            for b in range(B):
                sfb = sf_all[:, b, :]
                mkb = mk_all[:, b, :, :]
                for v in range(VC):
                    qt = qpool.tile([128, L], f32r)
                    nc.sync.dma_start(out=qt, in_=qf[b, v])
                    vv = vpool.tile([128, T, D + 1], bf)
                    nc.sync.dma_start(out=vv, in_=vp[b, v])

                    at = apool.tile([128, T, L], bf)
                    for g in range(T // 2):
                        ps = spsum.tile([128, 2, L], f32)
                        for hh in range(2):
                            jt = 2 * g + hh
                            nc.tensor.matmul(
                                ps[:, hh, :], sfb[:, _ts(jt, 128)], qt,
                                start=True, stop=True,
                            )
                        nc.scalar.activation(
                            at[:, 2 * g:2 * g + 2, :], ps,
                            mybir.ActivationFunctionType.Exp, scale=SCALE,
                        )
                        nc.vector.tensor_mul(
                            at[:, 2 * g:2 * g + 2, :],
                            at[:, 2 * g:2 * g + 2, :],
                            mkb[:, 2 * g:2 * g + 2, :],
                        )

                    og = opool.tile([128, T, D], f32)
                    for it in range(T):
                        ops = opsum.tile([128, D + 1], f32)
                        for jt in range(T):
                            nc.tensor.matmul(
                                ops, at[:, jt, _ts(it, 128)], vv[:, jt, :],
                                start=(jt == 0), stop=(jt == T - 1),
                            )
                        rc = rpool.tile([128, 1], f32)
                        nc.vector.reciprocal(rc, ops[:, D:D + 1])
                        nc.vector.tensor_scalar_mul(og[:, it, :], ops[:, 0:D], rc)
                    nc.sync.dma_start(
                        out=out[b, :, v, :].rearrange("(t p) d -> p t d", p=128),
                        in_=og,
                    )
    _split_waits(nc)
    return nc


_DENSE_CACHE = None


def _kernel_dense(query, key, value, label_arr):
    global _DENSE_CACHE
    if _DENSE_CACHE is None:
        _DENSE_CACHE = _build_dense()
    nc = _DENSE_CACHE

    query = np.asarray(query, dtype=F32)
    key = np.asarray(key, dtype=F32)
    value = np.asarray(value, dtype=F32)
    lab = np.asarray(label_arr)

    sum_tot = key.sum(axis=2)
    sT = np.ascontiguousarray(sum_tot.transpose(0, 2, 1))
    qT = np.ascontiguousarray(query.transpose(0, 2, 3, 1))

    v4 = value.reshape(B, T, 128, V, D).transpose(0, 3, 2, 1, 4)
    vp = np.empty((B, V, 128, T, D + 1), dtype=BF16)
    vp[..., :D] = v4.astype(BF16)
    vp[..., D] = np.ones((), dtype=BF16)

    labr = lab.reshape(B, T, 128)
    m = (labr[:, :, :, None] == lab[:, None, None, :])
    mkm = np.ascontiguousarray(m.transpose(0, 2, 1, 3)).astype(BF16)

    in_maps = []
    for c in range(N_CORES):
        sl = slice(c * VC, (c + 1) * VC)
        in_maps.append({
            "qf": np.ascontiguousarray(qT[:, sl]),
            "sf": sT,
            "vp": np.ascontiguousarray(vp[:, sl]),
            "mk": mkm,
        })
    res = run_bass_kernel_spmd(nc, in_maps, core_ids=list(range(N_CORES)))
    full = np.empty((B, L, V, D), dtype=F32)
    for c in range(N_CORES):
        full[:, :, c * VC:(c + 1) * VC, :] = res.results[c]["out"]
    return full
